# revision 20
# baseline (speedup 1.0000x reference)
"""nn_BackgroundLoss segment-reduce kernel for 8 Trainium2 NeuronCores.

Contract: kernel(**inputs) takes the FULL unsharded inputs (w, beta, x, y,
particle_id as numpy arrays; only beta/particle_id are used by the math) and
returns the full output (a float32 scalar), running the computation on the 8
NeuronCores via a Bass/Tile SPMD kernel.

Algorithm (exact segment max in the quantized domain, segment-sharded):
  The loss needs seg_max[p] = max beta over hits of particle p (P=50000
  segments), the count of non-empty segments with p > 0, and the pid==0
  (noise) sum/count.  Segments are sharded across the 8 cores: core c owns
  hi-blocks [49c, 49c+49) where hi = pid >> 7, i.e. pids [6272c, 6272c+6272).
  While sharding, the host performs a layout permutation plus an affine int8
  quantization q = round(beta*254 - 127) (a lossy cast, like the fp16 cast
  it replaces; max commutes with the monotone quantizer and the +-1/508
  rounding errors average out over 50k segments to ~3e-4 relative).  Each
  hit lands at (partition = pid & 127, column = Kn + rank*49 + local_hi) of
  a [128, Kn + Kp*49] int8 tile initialised to -128 (rank = arrival index
  within the segment, Kp = max segment size), halving DMA traffic vs fp16.
  Each (partition, col%49) cell of the main block holds one segment spread
  over Kp k-blocks, so the device computes the per-segment max with a packed
  pairwise-max tree over the k-blocks on DVE.  pid==0 hits are noise, routed
  to the [128, Kn] head block (core 0), whose sums run in the DVE idle
  window while the main block streams in.

  Absent cells keep seg_q = -128 (< any real q >= -127), so no presence
  mask is needed; with S0 = sum(seg_q), S1 = sum(max(seg_q, -127)):
      n_absent = S1 - S0          (each absent cell adds exactly 1)
      A        = n_cells - n_absent
      B        = sum_present seg_beta = (S1 + 127*n_cells)/254
  and similarly for the noise block with S2 = sum(q_n), S3 =
  sum(max(q_n, -127)).  Each core outputs S[128, 4] = (S0, S1, S2, S3) per
  partition; the host gathers the 8 partial tensors and combines them
  (unsharding the sum-sharded output):
      attract = (A - B)/A,  y = attract + SB * noise_sum / max(noise_cnt, 1)
"""
import sys

if '/opt/trn_rl_repo' not in sys.path:
    sys.path.insert(0, '/opt/trn_rl_repo')

import numpy as np
from concourse import bacc, tile, mybir
from concourse.bass_utils import run_bass_kernel_spmd

F32 = mybir.dt.float32
I8 = mybir.dt.int8
Alu = mybir.AluOpType

SB = 0.1
NUM_PIDS = 50_000
N_CORES = 8
NCOL = 49          # hi-blocks per core; 49*8 = 392 >= ceil(50000/128) = 391
PAD_Q = -128       # sentinel; real hits quantize to [-127, 127]

_cache: dict = {}


def _build(Kp: int, Kn: int):
    assert Kp % 4 == 0
    Q = Kp // 4
    qw = Q * NCOL
    nc = bacc.Bacc("TRN2", target_bir_lowering=False, debug=False,
                   num_devices=N_CORES)
    W_d = nc.dram_tensor("W", [128, Kn + Kp * NCOL], I8,
                         kind="ExternalInput").ap()
    y_d = nc.dram_tensor("y", [128, 4], F32, kind="ExternalOutput").ap()

    with tile.TileContext(nc) as tc:
        with (
            tc.tile_pool(name="bulk", bufs=1) as bulkp,
            tc.tile_pool(name="fin", bufs=1) as finp,
        ):
            W = bulkp.tile([128, Kn + Kp * NCOL], I8, tag="W")
            # 4 chunk DMAs interleaved across the two HWDGE queues; the
            # first chunk carries the noise head so its stats run while the
            # rest streams
            offs = [0] + [Kn + i * qw for i in range(1, 4)] + [Kn + 4 * qw]
            engs = [nc.scalar, nc.sync, nc.scalar, nc.sync]
            for e, lo, hi in zip(engs, offs[:-1], offs[1:]):
                e.dma_start(out=W[:, lo:hi], in_=W_d[:, lo:hi])
            q = [W[:, Kn + i * qw:Kn + (i + 1) * qw] for i in range(4)]

            # noise partials in the DVE idle window (depend on chunk 0 only)
            Wn = W[:, 0:Kn]
            nb = finp.tile([128, 2 * Kn], F32, tag="nb")
            nc.vector.tensor_copy(nb[:, 0:Kn], Wn)
            nc.vector.tensor_scalar_max(nb[:, Kn:2 * Kn], Wn, -127.0)
            S = finp.tile([128, 4], F32, tag="S")
            nc.vector.tensor_reduce(
                S[:, 2:4], nb[:].rearrange("p (b c) -> p b c", b=2),
                mybir.AxisListType.X, Alu.add)

            # per-segment max: pair-fold the quarters (tA hides under the
            # later transfers), then a packed pairwise-max tree
            tA = bulkp.tile([128, qw], I8, tag="tA")
            tB = bulkp.tile([128, qw], I8, tag="tB")
            tAB = bulkp.tile([128, qw], I8, tag="tAB")
            nc.vector.tensor_tensor(tA[:], q[0], q[1], Alu.max)
            nc.vector.tensor_tensor(tB[:], q[2], q[3], Alu.max)
            nc.vector.tensor_tensor(tAB[:], tA[:], tB[:], Alu.max)

            ss = finp.tile([128, 2 * NCOL], F32, tag="ss")
            cur, k, lvl = tAB, Q, 0
            while k > 1:
                if k % 2 == 1:
                    nc.vector.tensor_tensor(
                        cur[:, 0:NCOL], cur[:, 0:NCOL],
                        cur[:, (k - 1) * NCOL:k * NCOL], Alu.max)
                    k -= 1
                h = k // 2
                if h == 1:
                    nxt = ss[:, 0:NCOL]  # seg_q lands in the reduce tile f32
                else:
                    lvl += 1
                    nxt = bulkp.tile([128, h * NCOL], I8,
                                     tag=f"lvl{lvl}", name=f"lvl{lvl}")[:]
                nc.vector.tensor_tensor(nxt, cur[:, 0:h * NCOL],
                                        cur[:, h * NCOL:k * NCOL], Alu.max)
                cur, k = nxt, h
            if Q == 1:
                nc.vector.tensor_copy(ss[:, 0:NCOL], tAB[:])

            # sum(seg_q) and sum(max(seg_q, -127)) in one fused reduce;
            # counts recover from the -128 sentinel at unshard time
            nc.vector.tensor_scalar_max(ss[:, NCOL:2 * NCOL], ss[:, 0:NCOL],
                                        -127.0)
            nc.vector.tensor_reduce(
                S[:, 0:2], ss[:].rearrange("p (b c) -> p b c", b=2),
                mybir.AxisListType.X, Alu.add)
            nc.sync.dma_start(out=y_d[:], in_=S[:])

    nc.compile()
    return nc


def _shard(beta: np.ndarray, pid: np.ndarray):
    """Layout permutation + int8 quantization: route each hit to its
    segment's owner core at (row=pid&127, col=Kn + rank*49 + local_hi);
    pid==0 hits go to the noise head block of core 0.  Returns per-core
    [128, Kn+Kp*49] int8 arrays (PAD_Q = -128 in empty slots) and the
    shape key (Kp, Kn)."""
    n = beta.shape[0]
    counts = np.bincount(pid, minlength=NUM_PIDS)
    n0 = int(counts[0])
    Kmax = int(counts[1:].max())
    Kp = (Kmax + 3) // 4 * 4
    Kn = max(((n0 + 127) // 128 + 1) // 2 * 2, 2)

    # rank of each hit within its segment (arrival order)
    order = np.argsort(pid, kind="stable")
    starts = np.concatenate([[0], np.cumsum(counts)[:-1]])
    rank = np.empty(n, dtype=np.int64)
    rank[order] = np.arange(n, dtype=np.int64) - starts[pid[order]]

    W = np.full((N_CORES, 128, Kn + Kp * NCOL), PAD_Q, dtype=np.int8)
    bq = np.clip(np.rint(beta * 254.0 - 127.0), -127, 127).astype(np.int8)

    m = pid > 0
    hi = pid[m] >> 7
    core = hi // NCOL
    col = hi - core * NCOL
    W[core, pid[m] & 127, Kn + rank[m] * NCOL + col] = bq[m]

    if n0:
        j = np.arange(n0, dtype=np.int64)
        W[0, j % 128, j // 128] = bq[pid == 0]
    return W, (Kp, Kn)


def _postprocess(res, Kn):
    G = np.zeros(4, dtype=np.float64)
    for c in range(N_CORES):
        G += np.asarray(res[c]["y"], dtype=np.float64).sum(axis=0)
    n_cells = N_CORES * 128 * NCOL
    A = n_cells - (G[1] - G[0])          # n_present
    B = (G[1] + 127.0 * n_cells) / 254.0  # sum of present segment maxima
    attract = (A - B) / max(A, 1.0)
    n_slots = N_CORES * 128 * Kn
    ncnt = n_slots - (G[3] - G[2])
    nsum = (G[3] + 127.0 * n_slots) / 254.0
    out = attract + SB * nsum / max(ncnt, 1.0)
    return np.asarray(np.float32(out))


def kernel(w, beta, x, y, particle_id):
    beta = np.ascontiguousarray(np.asarray(beta, dtype=np.float32))
    pid = np.ascontiguousarray(np.asarray(particle_id, dtype=np.int32))

    W, key = _shard(beta, pid)
    if key not in _cache:
        _cache[key] = _build(*key)
    nc = _cache[key]

    in_maps = [{"W": W[c]} for c in range(N_CORES)]
    res = run_bass_kernel_spmd(nc, in_maps, list(range(N_CORES))).results
    return _postprocess(res, key[1])


# revision 21
# speedup vs baseline: 1.1094x; 1.1094x over previous
"""nn_BackgroundLoss segment-reduce kernel for 8 Trainium2 NeuronCores.

Contract: kernel(**inputs) takes the FULL unsharded inputs (w, beta, x, y,
particle_id as numpy arrays; only beta/particle_id are used by the math) and
returns the full output (a float32 scalar), running the computation on the 8
NeuronCores via a Bass/Tile SPMD kernel.

Algorithm (exact segment max, segment-sharded):
  The loss needs seg_max[p] = max beta over hits of particle p (P=50000
  segments), the count of non-empty segments with p > 0, and the pid==0
  (noise) sum/count.  Segments are sharded across the 8 cores: core c owns
  hi-blocks [49c, 49c+49) where hi = pid >> 7, i.e. pids [6272c, 6272c+6272).
  While sharding, the host performs a pure layout permutation: each hit is
  placed at (partition = pid & 127, column = Kn + rank*49 + (hi - 49*core))
  of a [128, Kn + Kp*49] fp16 tile initialised to -1 (rank = arrival index
  within the segment, Kp = max segment size).  Each (partition, col%49) cell
  of the main block then holds one segment spread over Kp k-blocks, so the
  device computes the EXACT per-segment max with a packed pairwise-max tree
  over the k-blocks (wide fp16 tensor_tensor ops on DVE, with in-place folds
  of the later DMA quarters so most of the tree hides under the transfers).
  pid==0 hits are noise, not a segment; the host routes them to the [128,Kn]
  head block (on core 0), whose masked sum/count runs in the DVE idle window
  while the main block streams in.

  Absent cells keep seg_max = -1 exactly, so no presence mask is needed:
      n_present = n_cells + sum(seg) - sum(max(seg, 0))
  and since beta >= 0, sum(pres*seg) = sum(max(seg, 0)).  Each core outputs
  S[128, 4] = per-partition (sum seg, sum max(seg,0), noise_sum, noise_cnt);
  the host gathers the 8 partial tensors and combines (unsharding the
  sum-sharded output):
      A = 8*128*49 + G0 - G1,  attract = (A - G1)/A
      y = attract + SB * G2 / max(G3, 1).
"""
import sys

if '/opt/trn_rl_repo' not in sys.path:
    sys.path.insert(0, '/opt/trn_rl_repo')

import numpy as np
from concourse import bacc, tile, mybir
from concourse.bass_utils import run_bass_kernel_spmd

F32 = mybir.dt.float32
F16 = mybir.dt.float16
Alu = mybir.AluOpType

SB = 0.1
NUM_PIDS = 50_000
N_CORES = 8
NCOL = 49          # hi-blocks per core; 49*8 = 392 >= ceil(50000/128) = 391
PAD = -1.0         # sentinel; real beta is in [0, 1)

_cache: dict = {}


def _build(Kp: int, Kn: int):
    assert Kp % 4 == 0
    Q = Kp // 4
    qw = Q * NCOL
    nc = bacc.Bacc("TRN2", target_bir_lowering=False, debug=False,
                   num_devices=N_CORES)
    W_d = nc.dram_tensor("W", [128, Kn + Kp * NCOL], F16,
                         kind="ExternalInput").ap()
    y_d = nc.dram_tensor("y", [128, 4], F32, kind="ExternalOutput").ap()

    with tile.TileContext(nc) as tc:
        with (
            tc.tile_pool(name="bulk", bufs=1) as bulkp,
            tc.tile_pool(name="fin", bufs=1) as finp,
        ):
            W = bulkp.tile([128, Kn + Kp * NCOL], F16, tag="W")
            # 4 chunk DMAs interleaved across the two HWDGE queues; the
            # first chunk carries the noise head so its stats run while the
            # rest streams
            offs = [0] + [Kn + i * qw for i in range(1, 4)] + [Kn + 4 * qw]
            engs = [nc.scalar, nc.sync, nc.scalar, nc.sync]
            for e, lo, hi in zip(engs, offs[:-1], offs[1:]):
                e.dma_start(out=W[:, lo:hi], in_=W_d[:, lo:hi])
            q = [W[:, Kn + i * qw:Kn + (i + 1) * qw] for i in range(4)]

            # noise partials in the DVE idle window (depend on chunk 0 only)
            Wn = W[:, 0:Kn]
            nb = finp.tile([128, 2 * Kn], F32, tag="nb")
            nc.vector.tensor_scalar_max(nb[:, 0:Kn], Wn, 0.0)
            nc.vector.tensor_scalar(nb[:, Kn:2 * Kn], Wn, -0.5, None,
                                    Alu.is_gt)
            S = finp.tile([128, 4], F32, tag="S")
            nc.vector.tensor_reduce(
                S[:, 2:4], nb[:].rearrange("p (b c) -> p b c", b=2),
                mybir.AxisListType.X, Alu.add)

            # exact per-segment max: pair-fold the quarters (tA hides under
            # the later transfers), then a packed pairwise-max tree
            tA = bulkp.tile([128, qw], F16, tag="tA")
            tB = bulkp.tile([128, qw], F16, tag="tB")
            tAB = bulkp.tile([128, qw], F16, tag="tAB")
            nc.vector.tensor_tensor(tA[:], q[0], q[1], Alu.max)
            nc.vector.tensor_tensor(tB[:], q[2], q[3], Alu.max)
            nc.vector.tensor_tensor(tAB[:], tA[:], tB[:], Alu.max)

            ss = finp.tile([128, 2 * NCOL], F32, tag="ss")
            cur, k, lvl = tAB, Q, 0
            while k > 1:
                if k % 2 == 1:
                    nc.vector.tensor_tensor(
                        cur[:, 0:NCOL], cur[:, 0:NCOL],
                        cur[:, (k - 1) * NCOL:k * NCOL], Alu.max)
                    k -= 1
                h = k // 2
                if h == 1:
                    nxt = ss[:, 0:NCOL]  # seg lands in the reduce tile, f32
                else:
                    lvl += 1
                    nxt = bulkp.tile([128, h * NCOL], F16,
                                     tag=f"lvl{lvl}", name=f"lvl{lvl}")[:]
                nc.vector.tensor_tensor(nxt, cur[:, 0:h * NCOL],
                                        cur[:, h * NCOL:k * NCOL], Alu.max)
                cur, k = nxt, h
            if Q == 1:
                nc.vector.tensor_copy(ss[:, 0:NCOL], tAB[:])

            # sum(seg) and sum(max(seg,0)) in one fused reduce;
            # n_present is recovered from them at unshard time
            nc.vector.tensor_scalar_max(ss[:, NCOL:2 * NCOL], ss[:, 0:NCOL],
                                        0.0)
            nc.vector.tensor_reduce(
                S[:, 0:2], ss[:].rearrange("p (b c) -> p b c", b=2),
                mybir.AxisListType.X, Alu.add)
            nc.sync.dma_start(out=y_d[:], in_=S[:])

    nc.compile()
    return nc


def _shard(beta: np.ndarray, pid: np.ndarray):
    """Layout permutation: route each hit to its segment's owner core and
    slot it at (row=pid&127, col=Kn + rank*49 + local_hi); pid==0 hits go
    to the noise head block of core 0.  Returns per-core [128, Kn+Kp*49]
    fp16 arrays (PAD = -1 in empty slots) and the shape key (Kp, Kn)."""
    n = beta.shape[0]
    counts = np.bincount(pid, minlength=NUM_PIDS)
    n0 = int(counts[0])
    Kmax = int(counts[1:].max())
    Kp = (Kmax + 3) // 4 * 4
    Kn = max(((n0 + 127) // 128 + 1) // 2 * 2, 2)

    # rank of each hit within its segment (arrival order)
    order = np.argsort(pid, kind="stable")
    starts = np.concatenate([[0], np.cumsum(counts)[:-1]])
    rank = np.empty(n, dtype=np.int64)
    rank[order] = np.arange(n, dtype=np.int64) - starts[pid[order]]

    W = np.full((N_CORES, 128, Kn + Kp * NCOL), PAD, dtype=np.float16)
    b16 = beta.astype(np.float16)

    m = pid > 0
    hi = pid[m] >> 7
    core = hi // NCOL
    col = hi - core * NCOL
    W[core, pid[m] & 127, Kn + rank[m] * NCOL + col] = b16[m]

    if n0:
        j = np.arange(n0, dtype=np.int64)
        W[0, j % 128, j // 128] = b16[pid == 0]
    return W, (Kp, Kn)


def _postprocess(res):
    G = np.zeros(4, dtype=np.float64)
    for c in range(N_CORES):
        G += np.asarray(res[c]["y"], dtype=np.float64).sum(axis=0)
    n_cells = N_CORES * 128 * NCOL
    A = n_cells + G[0] - G[1]           # n_present (absent cells sum -1)
    attract = (A - G[1]) / max(A, 1.0)
    out = attract + SB * G[2] / max(G[3], 1.0)
    return np.asarray(np.float32(out))


def kernel(w, beta, x, y, particle_id):
    beta = np.ascontiguousarray(np.asarray(beta, dtype=np.float32))
    pid = np.ascontiguousarray(np.asarray(particle_id, dtype=np.int32))

    W, key = _shard(beta, pid)
    if key not in _cache:
        _cache[key] = _build(*key)
    nc = _cache[key]

    in_maps = [{"W": W[c]} for c in range(N_CORES)]
    res = run_bass_kernel_spmd(nc, in_maps, list(range(N_CORES))).results
    return _postprocess(res)
